# revision 13
# baseline (speedup 1.0000x reference)
"""Trainium2 Bass kernel for nn_C4ByteNibbleVM.

The reference "soft VM" computes, per 32-bit word (4 bytes, one-hot f32
encoded), out = onehot(((a + b) mod 2^32) ^ a) bytewise with a ripple
carry.  With exact one-hot inputs every softmax in the reference is
saturated (logit gaps >= 20), so the reference output equals the exact
integer result to ~1e-7.  The kernel therefore:
  1. extracts byte indices from the one-hot inputs (dot with iota),
  2. does the 4-byte ripple-carry add + xor in integer arithmetic,
  3. re-expands to one-hot via is_equal against an iota row.
Data parallel over the word dimension: 8192 words per core x 8 cores.
"""

import numpy as np
import ml_dtypes

import concourse.bass as bass
import concourse.bacc as bacc
import concourse.mybir as mybir
from concourse.tile import TileContext
from concourse import bass_utils

B = 65536
NCORES = 8
BLOC = B // NCORES          # words per core
W = 4                       # 128-row chunks per iteration (512 words)
ROWS_PER_ITER = 128 * W
NITER = BLOC // (128 * W)

F32 = mybir.dt.float32
BF16 = mybir.dt.bfloat16
I32 = mybir.dt.int32
AX = mybir.AxisListType
OP = mybir.AluOpType


def build_kernel(n_words=BLOC, w=W, reps=1):
    """Build the per-core Bass module. n_words must divide into 128*w tiles.

    reps>1 repeats the whole computation (same I/O) inside one NEFF so
    steady-state per-rep time can be measured by differencing wall times.
    """
    rows_per_iter = 128 * w
    n_iter = n_words // rows_per_iter
    fd = 1024 * w  # free dim of one iteration tile

    nc = bacc.Bacc("TRN2", target_bir_lowering=False, debug=False)
    a_d = nc.dram_tensor("a", [n_words, 1024], F32, kind="ExternalInput")
    b_d = nc.dram_tensor("b", [n_words, 1024], F32, kind="ExternalInput")
    iota_d = nc.dram_tensor("iota", [128, fd], BF16, kind="ExternalInput")
    y_d = nc.dram_tensor("y", [n_words, 1024], F32, kind="ExternalOutput")

    # [n_iter, 128, w, 1024] views: iteration t covers words [rows_per_iter*t, ...)
    a_v = a_d[:].rearrange("(t s p) c -> t p s c", s=w, p=128)
    b_v = b_d[:].rearrange("(t s p) c -> t p s c", s=w, p=128)
    y_v = y_d[:].rearrange("(t s p) c -> t p s c", s=w, p=128)

    nseg = 4 * w  # one-hot segments per iteration tile

    with TileContext(nc) as tc:
        with (
            tc.tile_pool(name="cst", bufs=1) as cst,
            tc.tile_pool(name="ld", bufs=3) as ld,
            tc.tile_pool(name="mul", bufs=2) as mul,
            tc.tile_pool(name="idx", bufs=2) as idxp,
            tc.tile_pool(name="sm", bufs=2) as sm,
            tc.tile_pool(name="out", bufs=3) as outp,
        ):
            iota_sb = cst.tile([128, fd], BF16)
            nc.gpsimd.dma_start(iota_sb[:], iota_d[:])

            for t in [t for _ in range(reps) for t in range(n_iter)]:
                a_t = ld.tile([128, fd], BF16, tag="a")
                nc.gpsimd.dma_start(
                    a_t[:].rearrange("p (s c) -> p s c", c=1024), a_v[t]
                )
                b_t = ld.tile([128, fd], BF16, tag="b")
                nc.gpsimd.dma_start(
                    b_t[:].rearrange("p (s c) -> p s c", c=1024), b_v[t]
                )

                ma = mul.tile([128, fd], BF16, tag="m")
                nc.vector.tensor_tensor(ma[:], a_t[:], iota_sb[:], OP.mult)
                idxa = idxp.tile([128, nseg], F32, tag="ia")
                nc.vector.tensor_reduce(
                    idxa[:],
                    ma[:].rearrange("p (s c) -> p s c", c=256),
                    axis=AX.X,
                    op=OP.add,
                )
                mb = mul.tile([128, fd], BF16, tag="m")
                nc.vector.tensor_tensor(mb[:], b_t[:], iota_sb[:], OP.mult)
                idxb = idxp.tile([128, nseg], F32, tag="ib")
                nc.vector.tensor_reduce(
                    idxb[:],
                    mb[:].rearrange("p (s c) -> p s c", c=256),
                    axis=AX.X,
                    op=OP.add,
                )

                # ripple-carry add over byte positions i=0..3 (i inner in col)
                def bslice(ap, i):
                    return ap.rearrange("p (s i) -> p i s", i=4)[:, i : i + 1, :]

                csum = idxp.tile([128, nseg], F32, tag="cs")
                carry = None
                for i in range(4):
                    t0 = sm.tile([128, w], F32, tag=f"t0{i}")
                    nc.vector.tensor_tensor(
                        t0[:].rearrange("p (i s) -> p i s", i=1),
                        bslice(idxa[:], i),
                        bslice(idxb[:], i),
                        OP.add,
                    )
                    if carry is not None:
                        nc.vector.tensor_tensor(t0[:], t0[:], carry[:], OP.add)
                    cnew = sm.tile([128, w], F32, tag=f"c{i}")
                    nc.vector.tensor_scalar(cnew[:], t0[:], 256.0, None, OP.is_ge)
                    nc.vector.scalar_tensor_tensor(
                        bslice(csum[:], i),
                        cnew[:].rearrange("p (i s) -> p i s", i=1),
                        -256.0,
                        t0[:].rearrange("p (i s) -> p i s", i=1),
                        OP.mult,
                        OP.add,
                    )
                    carry = cnew

                # xor with operand a (int32), back to bf16 for compares
                s_i = sm.tile([128, nseg], I32, tag="si")
                nc.vector.tensor_copy(s_i[:], csum[:])
                a_i = sm.tile([128, nseg], I32, tag="ai")
                nc.vector.tensor_copy(a_i[:], idxa[:])
                x_i = sm.tile([128, nseg], I32, tag="xi")
                nc.vector.tensor_tensor(x_i[:], s_i[:], a_i[:], OP.bitwise_xor)
                x_f = sm.tile([128, nseg], F32, tag="xf")
                nc.vector.tensor_copy(x_f[:], x_i[:])

                out_t = outp.tile([128, fd], F32, tag="o")
                for j in range(nseg):
                    nc.vector.tensor_scalar(
                        out_t[:, j * 256 : (j + 1) * 256],
                        iota_sb[:, 0:256],
                        x_f[:, j : j + 1],
                        None,
                        OP.is_equal,
                    )
                nc.sync.dma_start(
                    y_v[t], out_t[:].rearrange("p (s c) -> p s c", c=1024)
                )

    nc.compile()
    return nc


_CACHED = {}


def _get_kernel(n_words=BLOC, w=W):
    key = (n_words, w)
    if key not in _CACHED:
        _CACHED[key] = build_kernel(n_words, w)
    return _CACHED[key]


def _iota_tile(w=W):
    row = np.tile(np.arange(256, dtype=np.float32), 4 * w)
    return np.broadcast_to(row, (128, 1024 * w)).astype(ml_dtypes.bfloat16)


def kernel(**inputs):
    a = np.asarray(inputs["a_bytes"], dtype=np.float32).reshape(B, 1024)
    b = np.asarray(inputs["b_bytes"], dtype=np.float32).reshape(B, 1024)
    nc = _get_kernel()
    iota = _iota_tile()
    in_maps = [
        {
            "a": a[c * BLOC : (c + 1) * BLOC],
            "b": b[c * BLOC : (c + 1) * BLOC],
            "iota": iota,
        }
        for c in range(NCORES)
    ]
    res = bass_utils.run_bass_kernel_spmd(nc, in_maps, core_ids=list(range(NCORES)))
    out = np.concatenate([res.results[c]["y"] for c in range(NCORES)], axis=0)
    return out.reshape(B, 4, 256)
